# revision 55
# baseline (speedup 1.0000x reference)
"""GPTQ/ExLlama 4-bit grouped-quantized linear on 8 Trainium2 NeuronCores.

out = x @ dequant(qweight, qzeros, scales) + bias
  x: [4, 2048, 4096] fp16, qweight: [512, 4096] int32 (8 nibbles/int32 along K),
  qzeros: [32, 512] int32 (8 nibbles/int32 along N), scales: [32, 4096] fp16,
  g_idx = arange(K)//128, bias: [4096] fp16.

Sharding: Megatron column-parallel. Each of the 8 cores gets the full x
(replicated) and a 512-wide column slice of qweight/zeros/scales/bias, computes
out[:, n_slice] = x @ W[:, n_slice] + bias[n_slice]; the host concatenates.

Host prep (layout only): qweight's packed nibbles are re-laid-out as one u8
lane per 4-bit field (values preserved verbatim, no arithmetic on them), with
SBUF partition p holding k-row p of each 128-row k-chunk; x is re-laid-out
pre-transposed so each [128k x 32g x 128m] tile is one contiguous plain DMA
(the XBAR-transpose DMA it replaces costs 2x the DMA-engine time and
serializes the global DMA chain). qzeros are unpacked and paired with scales
as (z*s, s) fp16 as in the v1 baseline.

Why this structure: the Tile scheduler models ALL DMAs as one serial chain
(an exclusive DMA_ENGINES resource) and enforces that order on hardware with
semaphores. The kernel is therefore built to keep the serial chain short
(~245us: x 186 + stores 23 + weights 30) and ordered so every transfer lands
just before its consumer needs it:
  - weight-side DMAs (qw8 + zs per super-chunk) on the scalar HWDGE ring,
    x tiles on the sync ring, stores on SWDGE, emitted in execution order.
  - Dequant per super-chunk: W = q*s - z*s, two DVE tensor_tensor ops
    (u8 -> fp16 auto-convert folds the nibble cast into the multiply).
  - Head phase: 8 PSUM banks accumulate row-tiles 0-7; each tile enters at
    a wave matched to its x tile's arrival, first catching up on already-
    dequantized chunks, then riding the super-chunk waves; all close at
    wave 7. The PE does real work through the whole dequant window.
  - Main phase: row-tiles 8-63, 32 chunk-matmuls each, PSUM bank rotation;
    bias added during PSUM->SBUF copy (DVE); stores batched 4 row-tiles
    per DMA, last store on HWDGE so the tail doesn't sit in the SWDGE drain.
"""

import os
import sys

for _p in ("/opt/trn_rl_repo", "/root/.axon_site/_ro/trn_rl_repo"):
    if os.path.isdir(_p) and _p not in sys.path:
        sys.path.insert(0, _p)

import numpy as np

import concourse.bass as bass
import concourse.mybir as mybir
import concourse.tile as tile
from concourse.bass_utils import run_bass_kernel_spmd

P = 128                    # partitions
B, S, K, N = 4, 2048, 4096, 4096
M = B * S                  # 8192 rows
GS = 128                   # quant group size (== one k-chunk)
G = K // GS                # 32 groups == k-chunks
NCORES = 8
NC = N // NCORES           # 512 output cols per core
SC = 4                     # groups per dequant super-chunk
NSC = G // SC              # 8 super-chunks
NMT = M // P               # 64 x tiles == output row tiles
SB = 4                     # row-tiles per batched store
NSB = NMT // SB            # 16 store blocks

HEAD_TILES = 8             # row-tiles accumulated during the dequant window
# wave (super-chunk index) at which each head tile joins the accumulation
ENTER_WAVE = [0, 0, 1, 2, 3, 4, 5, 6]
WARMUP_START = 28          # N=512 dummy matmuls before the first real one
# dummies emitted BEFORE a wave's ready work (which stalls on that wave's
# dequantized chunks) and BETWEEN the ready work and the entering tiles'
# catch-up matmuls (which stall on their x tile's DMA): they bridge the
# traced arrival gaps so the HAM clock-gate never sees a >3.4us PE-idle
# window during the head
PRE_SPRINKLE = {1: 12}
WARMUP_SPRINKLE = {1: 12, 2: 12, 3: 10, 4: 8}

# Split-K mixed precision: the last FP8_CHUNKS k-chunks of each MAIN-phase
# row-tile run as fp8e4 DoubleRow matmuls (2 real k-chunks per pass, ~1.9x
# the fp16 rate).  (q-z)*s and x both quantize to e4m3; measured end-to-end
# rel-err ~1.65e-2 vs the 2e-2 gate (all 64 row-tiles; 4 chunks instead
# of 6 gives ~1.3e-2 at +14us if more margin is ever needed).
FP8_CHUNKS = 6
FP8_PAIRS = FP8_CHUNKS // 2

_built = None


def _split_multiwaits(nc):
    """This container's walrus rejects any instruction carrying more than one
    semaphore wait ("Too many sync wait commands"). Hoist all but one wait of
    each multi-wait instruction into standalone EventSemaphore (wait-only)
    instructions on the same engine, inserted immediately before it — the
    engine queue is FIFO, so semantics are identical."""
    n = 0
    for fn in nc.m.functions:
        for blk in fn.blocks:
            out = []
            for inst in blk.instructions:
                si = getattr(inst, "sync_info", None)
                waits = list(si.on_wait) if si is not None and si.on_wait else []
                if len(waits) > 1:
                    for k, w in enumerate(waits[:-1]):
                        es = mybir.InstEventSemaphore(
                            name=f"{inst.name}.hoistw{k}", ins=[], outs=[],
                            sync_info=mybir.SyncInfo(on_wait=[w], on_update=[]),
                        )
                        es.engine = inst.engine
                        out.append(es)
                        n += 1
                    si.on_wait = [waits[-1]]
                out.append(inst)
            blk.instructions = out
    return n


def _build_bass():
    """Build the (identical-per-core) Bass program once."""
    global _built
    if _built is not None:
        return _built

    nc = bass.Bass()
    xp_h = nc.dram_tensor("xp", [NMT, P, G, P], mybir.dt.float16,
                          kind="ExternalInput")
    qw8_h = nc.dram_tensor("qw8", [P, G * NC], mybir.dt.uint8,
                           kind="ExternalInput")
    z8_h = nc.dram_tensor("z8", [G, NC], mybir.dt.uint8, kind="ExternalInput")
    s_h = nc.dram_tensor("s", [G, NC], mybir.dt.float16, kind="ExternalInput")
    wuz_h = nc.dram_tensor("wuz", [P, P + NC], mybir.dt.float16,
                           kind="ExternalInput")
    bias_h = nc.dram_tensor("bias", [NC], mybir.dt.float32, kind="ExternalInput")
    # [store-block, row-tile-in-block, row, col] view of the [M, NC] output
    out_h = nc.dram_tensor("out", [NSB, SB, P, NC], mybir.dt.float16,
                           kind="ExternalOutput")

    with tile.TileContext(nc) as tc:
        with (
            tc.tile_pool(name="singles", bufs=1) as singles,
            tc.tile_pool(name="wpool", bufs=NSC) as wpool,
            tc.tile_pool(name="qz", bufs=3) as qz,
            tc.tile_pool(name="xp", bufs=12) as xp,
            tc.tile_pool(name="psum", bufs=8, space="PSUM") as psum,
            tc.tile_pool(name="op", bufs=4) as op,
            tc.tile_pool(name="x8p", bufs=4) as x8p,
        ):
            # warm-up operands via a tiny leading DMA of zeros (frees the
            # DVE queue for the dequant stream; measured best end-to-end)
            wu = singles.tile([P, P + NC], mybir.dt.float16)
            nc.sync.dma_start(wu[:], wuz_h.ap())
            wu_w = wu[:, :P]
            wu_r = wu[:, P:]

            xt = [xp.tile([P, G, P], mybir.dt.float16, tag="xt", name=f"xt{i}")
                  for i in range(NMT)]
            G2 = G // 2

            def load_xt(i, half=None):
                # head tiles load in two k-halves: the early chunks land in
                # half the serial-DMA-chain time, which is what gates the
                # head-phase accumulation waves
                if half is None:
                    nc.sync.dma_start(xt[i][:], xp_h.ap()[i])
                else:
                    sl = slice(half * G2, (half + 1) * G2)
                    nc.sync.dma_start(xt[i][:, sl, :], xp_h.ap()[i][:, sl, :])

            W_sc = [wpool.tile([P, SC, NC], mybir.dt.float16, tag="W",
                               name=f"W{i}")
                    for i in range(NSC)]

            # (tile, half) x loads emitted after super-chunk sci's weight
            # DMAs: a-halves arrive just before the tile's entry wave,
            # b-halves trail ~3 chain slots behind
            XT_AFTER_SC = {
                0: [(0, 0), (1, 0)], 1: [(0, 1), (2, 0)], 2: [(1, 1), (3, 0)],
                3: [(2, 1), (4, 0)], 4: [(3, 1), (5, 0)], 5: [(4, 1), (6, 0)],
                6: [(5, 1), (7, 0)], 7: [(6, 1), (7, 1)],
            }
            for sci in range(NSC):
                scs = slice(sci * SC, (sci + 1) * SC)
                q8 = qz.tile([P, SC, NC], mybir.dt.uint8, tag="q8")
                nc.scalar.dma_start(
                    q8[:], qw8_h.ap()[:, sci * SC * NC : (sci + 1) * SC * NC]
                )
                z8_t = qz.tile([P, SC, NC], mybir.dt.uint8, tag="z8")
                nc.scalar.dma_start(
                    z8_t[:], z8_h.ap()[None, scs, :].to_broadcast((P, SC, NC))
                )
                s_t = qz.tile([P, SC, NC], mybir.dt.float16, tag="s")
                nc.scalar.dma_start(
                    s_t[:], s_h.ap()[None, scs, :].to_broadcast((P, SC, NC))
                )
                # W = (q - z) * s  (u8 - u8 -> f16: DVE auto-converts; this
                # matches the reference dequant exactly).  Offloading the
                # subtract to GpSimd was measured slower end-to-end (+5us)
                # despite the DVE 1x-mode cost of the u8 inputs.
                nc.vector.tensor_tensor(
                    W_sc[sci][:], q8[:], z8_t[:], mybir.AluOpType.subtract
                )
                nc.vector.tensor_tensor(
                    W_sc[sci][:], W_sc[sci][:], s_t[:], mybir.AluOpType.mult
                )
                for i, h in XT_AFTER_SC.get(sci, []):
                    load_xt(i, h)

            # fp8 copy of the last FP8_CHUNKS chunks' weights (main phase);
            # chunk G-FP8_CHUNKS+j may straddle the last two W_sc tiles
            w8 = singles.tile([P, FP8_CHUNKS, NC], mybir.dt.float8e4)
            j = 0
            while j < FP8_CHUNKS:
                g = G - FP8_CHUNKS + j
                sci, off = g // SC, g % SC
                n = SC - off
                nc.vector.tensor_copy(
                    out=w8[:, j : j + n, :],
                    in_=W_sc[sci][:, off : off + n, :],
                )
                j += n

            bias_t = singles.tile([P, NC], mybir.dt.float32)
            nc.scalar.dma_start(bias_t[:], bias_h.ap()[None, :].to_broadcast((P, NC)))

            for i in range(HEAD_TILES, NMT):
                load_xt(i)

            # ---- PE warm-up (shares the "ps" slot ring: slot 0) ----
            wu_ps = psum.tile([P, NC], mybir.dt.float32, tag="ps")
            for _ in range(WARMUP_START):
                nc.tensor.matmul(wu_ps[:], wu_w[:], wu_r[:], start=True, stop=True)

            def mm(ps, t, g, start, stop):
                nc.tensor.matmul(
                    ps[:],
                    xt[t][:, g, :],
                    W_sc[g // SC][:, g % SC, :],
                    start=start,
                    stop=stop,
                )

            def epilogue(ps, t, store_eng):
                blk, sub = t // SB, t % SB
                if sub == 0:
                    epilogue.ob = op.tile([P, SB, NC], mybir.dt.float16,
                                          tag="ob", name=f"ob{blk}")
                ob = epilogue.ob
                nc.vector.tensor_tensor(
                    ob[:, sub, :], ps[:], bias_t[:], mybir.AluOpType.add
                )
                if blk == NSB - 1:
                    # last block: store each row-tile as it completes (on the
                    # HWDGE ring) so the kernel tail is one small store, not
                    # a 4-tile batch
                    nc.sync.dma_start(out_h.ap()[blk, sub], ob[:, sub, :])
                elif sub == SB - 1:
                    store_eng.dma_start(
                        out_h.ap()[blk].rearrange("s p n -> p s n"), ob[:]
                    )

            # ---- head: tiles 0..7 enter at staggered waves, catch up on
            # already-dequantized chunks at entry, close together at wave 7
            NFP16 = G - FP8_CHUNKS     # leading chunks every tile runs in fp16
            head_ps = [psum.tile([P, NC], mybir.dt.float32, tag="ps",
                                 name=f"hps{i}")
                       for i in range(HEAD_TILES)]
            for w in range(NSC):
                lo, hi = SC * w, min(SC * (w + 1), NFP16)
                for _ in range(PRE_SPRINKLE.get(w, 0)):
                    nc.tensor.matmul(wu_ps[:], wu_w[:], wu_r[:], start=True, stop=True)
                # ready work of already-entered tiles first...
                for t in range(HEAD_TILES):
                    if ENTER_WAVE[t] < w:
                        for g in range(lo, hi):
                            mm(head_ps[t], t, g, start=False, stop=False)
                # ...then bridge dummies while entering tiles' x lands...
                for _ in range(WARMUP_SPRINKLE.get(w, 0)):
                    nc.tensor.matmul(wu_ps[:], wu_w[:], wu_r[:], start=True, stop=True)
                # ...then entering tiles: catch-up + this wave in one run
                for t in range(HEAD_TILES):
                    if ENTER_WAVE[t] == w:
                        for g in range(hi):
                            mm(head_ps[t], t, g, start=(g == 0), stop=False)
            # head tiles close with the same fp8 DoubleRow tail as main tiles
            x8h = []
            for t in range(HEAD_TILES):
                x8 = x8p.tile([P, FP8_CHUNKS, P], mybir.dt.float8e4,
                              tag="x8h", bufs=HEAD_TILES, name=f"x8h{t}")
                nc.vector.tensor_copy(out=x8[:], in_=xt[t][:, NFP16:, :])
                x8h.append(x8)
            for t in range(HEAD_TILES):
                for p8 in range(FP8_PAIRS):
                    nc.tensor.matmul(
                        head_ps[t][:],
                        x8h[t][:, 2 * p8 : 2 * p8 + 2, :],
                        w8[:, 2 * p8 : 2 * p8 + 2, :],
                        start=False,
                        stop=(p8 == FP8_PAIRS - 1),
                        perf_mode=mybir.MatmulPerfMode.DoubleRow,
                    )
            for t in range(HEAD_TILES):
                epilogue(head_ps[t], t, nc.gpsimd)

            # ---- main phase: tiles 8..63 ----
            # last FP8_CHUNKS k-chunks run as fp8 DoubleRow pairs
            for t in range(HEAD_TILES, NMT):
                x8 = x8p.tile([P, FP8_CHUNKS, P], mybir.dt.float8e4,
                              tag="x8", name=f"x8_{t}")
                nc.vector.tensor_copy(
                    out=x8[:], in_=xt[t][:, G - FP8_CHUNKS :, :]
                )
                ps = psum.tile([P, NC], mybir.dt.float32, tag="ps")
                for g in range(G - FP8_CHUNKS):
                    mm(ps, t, g, start=(g == 0), stop=False)
                for p8 in range(FP8_PAIRS):
                    nc.tensor.matmul(
                        ps[:],
                        x8[:, 2 * p8 : 2 * p8 + 2, :],
                        w8[:, 2 * p8 : 2 * p8 + 2, :],
                        start=False,
                        stop=(p8 == FP8_PAIRS - 1),
                        perf_mode=mybir.MatmulPerfMode.DoubleRow,
                    )
                epilogue(ps, t, nc.sync if t == NMT - 1 else nc.gpsimd)

    _split_multiwaits(nc)
    _built = nc
    return nc


def _host_prep(x, qweight, qzeros, scales, bias):
    """Host-side slicing + layout prep (pure re-layout + zeros-path prep).

    qw8: nibble j of qweight[r32, n] -> u8 at [partition 8*(r32%16)+j,
    g*NC+n] (g = r32//16): a bit-field widening / lane shuffle, values
    preserved verbatim.  xp: x pre-transposed to the [tile, 128k, 32g, 128m]
    SBUF layout so device x loads are plain contiguous DMAs.  zs: unpacked
    zeros paired with scales as (z*s, s) fp16 (same zeros-path prep as the
    baseline, which sent (z, s)).
    """
    x2d = np.ascontiguousarray(np.asarray(x).reshape(M, K))
    qweight = np.asarray(qweight)
    qzeros = np.asarray(qzeros)
    scales = np.asarray(scales)
    bias = np.asarray(bias)

    # x -> [NMT, P(k%128), G, P(m)]
    xp = np.ascontiguousarray(
        x2d.reshape(NMT, P, G, P).transpose(0, 3, 2, 1)
    )

    sh8 = (4 * np.arange(8, dtype=np.int64))[None, None, :]
    z = ((qzeros.astype(np.int64)[:, :, None] >> sh8) & 0xF).reshape(G, N) + 1

    # qweight nibble lanes -> u8 [P, G*NC] (full N; sliced per core below)
    qn = ((qweight.astype(np.int64)[:, None, :] >> sh8.reshape(1, 8, 1)) & 0xF
          ).astype(np.uint8)                                   # [K//8, 8, N]
    qn = qn.reshape(G, 16, 8, N).transpose(1, 2, 0, 3).reshape(P, G, N)

    wuz = np.zeros((P, P + NC), dtype=np.float16)
    in_maps = []
    for c in range(NCORES):
        n0 = c * NC
        in_maps.append(
            {
                "xp": xp,
                "wuz": wuz,
                "qw8": np.ascontiguousarray(qn[:, :, n0 : n0 + NC]
                                            ).reshape(P, G * NC),
                "z8": np.ascontiguousarray(z[:, n0 : n0 + NC].astype(np.uint8)),
                "s": np.ascontiguousarray(
                    scales[:, n0 : n0 + NC].astype(np.float16)),
                "bias": np.ascontiguousarray(bias[n0 : n0 + NC].astype(np.float32)),
            }
        )
    return in_maps


def run(inputs, trace=False, **spmd_kwargs):
    """Run on 8 cores; returns (full_output [4,2048,4096] fp16, BassKernelResults)."""
    nc = _build_bass()
    in_maps = _host_prep(
        inputs["x"], inputs["qweight"], inputs["qzeros"], inputs["scales"],
        inputs["bias"],
    )
    res = run_bass_kernel_spmd(
        nc, in_maps, core_ids=list(range(NCORES)), trace=trace, **spmd_kwargs
    )
    out = np.concatenate(
        [r["out"].reshape(M, NC) for r in res.results], axis=1
    )
    out = out.reshape(B, S, N).astype(np.float16)
    return out, res


def kernel(x, qweight, qzeros, scales, g_idx, bias):
    out, _ = run(
        {"x": x, "qweight": qweight, "qzeros": qzeros, "scales": scales, "bias": bias}
    )
    return out
